# revision 18
# baseline (speedup 1.0000x reference)
"""Causal ReLU-attention block (qkv proj + per-head attention) on 8 trn2 cores.

Sharding: pure data-parallel over batch (B=8 -> 1 batch element per core).

Schedule (single TileContext, PE-bound):
  1. qk projection tiles for head pairs 0,1  (q0,k0,q1,k1)
  2. v projection (all t-tiles)
  3. attention pairs (0,1)  interleaved with qk tiles for pairs 2,3
  4. attention pairs (2,3)  interleaved with qk tiles for pairs 4,5
  5. attention pairs (4,5)
PSUM->SBUF evictions are load-balanced across ACT and DVE via a cost model;
causal masking of diagonal strips runs on gpsimd. Host side: weights are
pre-laid-out tile-major so every DMA is a large per-partition-contiguous
transfer; y is returned transposed in bf16 and cast on host.
"""

import sys
from contextlib import ExitStack

sys.path.insert(0, "/opt/trn_rl_repo")

import ml_dtypes
import numpy as np

import concourse.bass as bass
import concourse.tile as tile
from concourse import bacc, bass_utils, mybir

P = 128
QW = 512  # t_q chunk width (PSUM bank = 512 fp32)

BF16 = mybir.dt.bfloat16
F32 = mybir.dt.float32
AF = mybir.ActivationFunctionType
ALU = mybir.AluOpType


def build_module(T=1024, C=768, H=12, n_cores=8):
    """Build + compile the per-core Bass module (same program on all cores)."""
    hd = C // H
    assert hd == 64 and H % 2 == 0 and C % P == 0 and T % QW == 0
    CT = C // P            # contraction tiles over C
    TT = T // P            # t tiles
    NQC = T // QW          # q chunks
    VW = C // 2            # v output free-dim chunk width (<=512)
    NHP = H // 2           # head pairs
    scale = 1.0 / float(np.sqrt(hd))

    nc = bacc.Bacc("TRN2", target_bir_lowering=False, debug=False,
                   num_devices=n_cores)

    # q/k weights, tile-major interleaved [q0,k0,q1,k1,...]: [P, 12, CT, P]
    wqk = nc.dram_tensor("wqk", [P, 2 * C * CT], BF16,
                         kind="ExternalInput").ap()
    # v weights [P, CT, C]
    wv = nc.dram_tensor("wv", [P, CT * C], BF16, kind="ExternalInput").ap()
    xL = nc.dram_tensor("xL", [P, CT * T], BF16, kind="ExternalInput").ap()
    bqk = nc.dram_tensor("bqk", [P, 2 * CT], F32, kind="ExternalInput").ap()
    bv = nc.dram_tensor("bv", [P, C], F32, kind="ExternalInput").ap()
    yT = nc.dram_tensor("yT", [C, T], BF16, kind="ExternalOutput").ap()

    wqk3 = wqk.rearrange("p (j ct m) -> p j ct m", j=2 * CT, ct=CT)
    wv3 = wv.rearrange("p (ct o) -> p ct o", ct=CT)
    xL3 = xL.rearrange("p (ct t) -> p ct t", ct=CT)

    with tile.TileContext(nc) as tc, ExitStack() as ctx:
        const = ctx.enter_context(tc.tile_pool(name="const", bufs=1))
        psum = ctx.enter_context(tc.tile_pool(name="psum", bufs=3, space="PSUM"))
        ypsum = ctx.enter_context(tc.tile_pool(name="ypsum", bufs=2, space="PSUM"))
        scb = ctx.enter_context(tc.tile_pool(name="scb", bufs=12))
        ysb = ctx.enter_context(tc.tile_pool(name="ysb", bufs=3))

        wqk_sb = const.tile([P, 2 * CT, CT, P], BF16)
        wv_sb = const.tile([P, CT, C], BF16)
        xt_sb = const.tile([P, CT, T], BF16)
        bqk_sb = const.tile([P, 2 * CT], F32)
        bv_sb = const.tile([P, 2, VW], F32)

        # input staging in need-priority classes, byte-balanced across the
        # two HWDGE queues (the 16 shared DMA engines saturate ~355GB/s
        # aggregate, so order is what matters, not more queues).
        nc.scalar.dma_start(bqk_sb[:], bqk[:])
        nc.sync.dma_start(xt_sb[:, 2:4, 0:QW], xL3[:, 2:4, 0:QW])
        nc.scalar.dma_start(wqk_sb[:, 0:1], wqk3[:, 0:1])             # q0
        nc.sync.dma_start(xt_sb[:, 4:6, 0:QW], xL3[:, 4:6, 0:QW])
        nc.scalar.dma_start(xt_sb[:, 0:2, 0:QW], xL3[:, 0:2, 0:QW])
        nc.sync.dma_start(wqk_sb[:, 1:2], wqk3[:, 1:2])               # k0
        nc.scalar.dma_start(wqk_sb[:, 2:3], wqk3[:, 2:3])             # q1
        nc.sync.dma_start(wqk_sb[:, 3:4], wqk3[:, 3:4])               # k1
        nc.scalar.dma_start(wqk_sb[:, 5:6], wqk3[:, 5:6])             # k2
        nc.sync.dma_start(wqk_sb[:, 4:5], wqk3[:, 4:5])               # q2
        nc.scalar.dma_start(xt_sb[:, 0:3, QW:T], xL3[:, 0:3, QW:T])
        nc.sync.dma_start(xt_sb[:, 3:6, QW:T], xL3[:, 3:6, QW:T])
        nc.scalar.dma_start(wv_sb[:, :, 0:VW], wv3[:, :, 0:VW])
        nc.sync.dma_start(wv_sb[:, :, VW:C], wv3[:, :, VW:C])
        nc.scalar.dma_start(wqk_sb[:, 6:8], wqk3[:, 6:8])             # q3,k3
        nc.sync.dma_start(wqk_sb[:, 8:10], wqk3[:, 8:10])             # q4,k4
        nc.scalar.dma_start(wqk_sb[:, 10:12], wqk3[:, 10:12])         # q5,k5
        nc.sync.dma_start(bv_sb[:], bv.rearrange("p (oc v) -> p oc v", oc=2))

        # PE warm-up: HAM starts at K=4/8 (1.2 GHz) and needs ~3.4us of
        # sustained activity to unthrottle; burn the initial DMA wait on
        # dummy matmuls so real chains run at 2.4 GHz from the start.
        wu_src = const.tile([P, 256], BF16)
        nc.gpsimd.memset(wu_src[:], 0.0)
        wu_ps = ypsum.tile([P, QW], F32, tag="y", name="wu_ps")
        for _ in range(18):
            nc.tensor.matmul(wu_ps[0:1, 0:256], wu_src[:, 0:1],
                             wu_src[:], start=True, stop=True)

        qkT = const.tile([P, 2 * CT, T], BF16)   # tile j: q_h=j//2 (even) / k (odd)
        vsb = const.tile([P, TT, C], BF16)       # v in natural [t, o] layout

        # ---- ACT/DVE eviction load balancer (ns cost model, trn2 errata) ----
        est = {"a": 0.0, "d": 0.0}

        def _pick(fd):
            a_cost = (172.0 + fd) / 1.2
            d_cost = (120.0 + fd) / 0.96
            if est["a"] + a_cost <= est["d"] + d_cost:
                est["a"] += a_cost
                return "a"
            est["d"] += d_cost
            return "d"

        def relu_evict(dst, src, fd):
            # relu(scale * s): PSUM -> SBUF bf16
            if _pick(fd) == "a":
                nc.scalar.activation(dst, src, AF.Relu, scale=scale)
            else:
                nc.vector.tensor_scalar(dst, src, scale, 0.0, ALU.mult, ALU.max)

        def y_evict(dst, src):
            if _pick(QW) == "a":
                nc.scalar.activation(dst, src, AF.Copy)
            else:
                nc.vector.tensor_copy(dst, src)

        def qk_drain(j, ps):
            flat = ps[:, :NQC].rearrange("p a b -> p (a b)")
            if _pick(NQC * QW) == "a":
                nc.scalar.activation(qkT[:, j], flat, AF.Identity,
                                     bias=bqk_sb[:, j:j + 1])
            else:
                nc.vector.tensor_tensor(
                    qkT[:, j], flat,
                    bqk_sb[:, j:j + 1].to_broadcast((P, NQC * QW)), ALU.add)

        # ---- projections ----
        def qk_chain(j, qc, ps, slot=None):
            slot = qc if slot is None else slot
            for ct in range(CT):
                nc.tensor.matmul(
                    ps[:, slot],
                    wqk_sb[:, j, ct],
                    xt_sb[:, ct, qc * QW:(qc + 1) * QW],
                    start=(ct == 0), stop=(ct == CT - 1),
                )

        def qk_drain_half(j, qc, ps):
            dst = qkT[:, j, qc * QW:(qc + 1) * QW]
            if _pick(QW) == "a":
                nc.scalar.activation(dst, ps[:, 0], AF.Identity,
                                     bias=bqk_sb[:, j:j + 1])
            else:
                nc.vector.tensor_tensor(
                    dst, ps[:, 0],
                    bqk_sb[:, j:j + 1].to_broadcast((P, QW)), ALU.add)

        def emit_qk_half(j, qc):
            # one qc chunk per PSUM tile so the qc=0 chains of all four
            # phase-1 tiles can run before x's second half has landed
            ps = psum.tile([P, 2, QW], F32, tag="blk", name="qk_ps")
            qk_chain(j, qc, ps, slot=0)
            qk_drain_half(j, qc, ps)

        def emit_v(tt, ocs=(0, 1)):
            ps = psum.tile([P, 2, QW], F32, tag="blk", name="v_ps")
            for ct in range(CT):
                for oc in ocs:
                    nc.tensor.matmul(
                        ps[:, oc, :VW],
                        xt_sb[:, ct, tt * P:(tt + 1) * P],
                        wv_sb[:, ct, oc * VW:(oc + 1) * VW],
                        start=(ct == 0), stop=(ct == CT - 1),
                    )
            nc.vector.tensor_tensor(
                vsb[:, tt].rearrange("p (oc v) -> p oc v", oc=2),
                ps[:, :, :VW], bv_sb[:], ALU.add)
            est["d"] += (120.0 + C) / 0.96

        # ---- attention ----
        def attention_closures(hp):
            """Score / att@v closures per block step for one head pair; the
            interleaver runs att@v LAG super-steps behind its scores."""
            items = []
            for qc in range(NQC):
                kb_hi = min((qc * QW + QW - 1) // P, TT - 1)
                for kb in range(kb_hi + 1):
                    items.append((qc, kb, kb_hi))
            state = {"s": {}, "y": {}}
            sc_fns, av_fns = [], []

            def sc(i, qc, kb, kb_hi):
                delta = max(kb * P - qc * QW, 0)   # first valid t_q col
                sp = psum.tile([P, 2, QW], F32, tag="blk", name="s_ps")
                for h, ppos in ((0, (0, 0)), (1, (64, 0))):
                    nc.tensor.matmul(
                        sp[:, h, delta:QW],
                        qkT[h * 64:(h + 1) * 64, 2 * hp + 1,
                            kb * P:(kb + 1) * P],
                        qkT[h * 64:(h + 1) * 64, 2 * hp,
                            qc * QW + delta:(qc + 1) * QW],
                        start=True, stop=True, tile_position=ppos,
                    )
                s = scb.tile([P, 2, QW], BF16, tag="s")
                relu_evict(s[:, :, delta:QW], sp[:, :, delta:QW],
                           2 * (QW - delta))
                if kb * P >= qc * QW:   # diagonal strip: causal mask on the
                    # first P cols only (row p can only mask j' < p < P)
                    nc.gpsimd.affine_select(
                        s[:, :, delta:delta + P],
                        s[:, :, delta:delta + P],
                        pattern=[[0, 2], [1, P]],
                        compare_op=ALU.is_ge, fill=0.0,
                        base=0, channel_multiplier=-1,
                    )
                state["s"][i] = s

            def av(i, qc, kb, kb_hi):
                if kb == 0:
                    state["y"][qc] = ypsum.tile([P, QW], F32, tag="y",
                                                name="yp")
                yp = state["y"][qc]
                delta = max(kb * P - qc * QW, 0)
                s = state["s"].pop(i)
                # the two heads accumulate into disjoint partition ranges of
                # one bank; each runs its own start/stop group
                nc.tensor.matmul(
                    yp[0:64, delta:QW], vsb[:, kb, hp * P:hp * P + 64],
                    s[:, 0, delta:QW],
                    start=(kb == 0), stop=(kb == kb_hi),
                    tile_position=(0, 0), skip_group_check=True,
                )
                nc.tensor.matmul(
                    yp[64:128, delta:QW],
                    vsb[:, kb, hp * P + 64:hp * P + 128],
                    s[:, 1, delta:QW],
                    start=(kb == 0), stop=(kb == kb_hi),
                    tile_position=(0, 64), skip_group_check=True,
                )
                if kb == kb_hi:
                    yp = state["y"].pop(qc)
                    yt = ysb.tile([P, QW], BF16, tag="yt")
                    y_evict(yt[:], yp[:])
                    nc.sync.dma_start(
                        yT[hp * P:(hp + 1) * P, qc * QW:(qc + 1) * QW],
                        yt[:])

            for i, (qc, kb, kb_hi) in enumerate(items):
                sc_fns.append(
                    lambda i=i, qc=qc, kb=kb, kb_hi=kb_hi: sc(i, qc, kb, kb_hi))
                av_fns.append(
                    lambda i=i, qc=qc, kb=kb, kb_hi=kb_hi: av(i, qc, kb, kb_hi))
            return sc_fns, av_fns

        def run_group(pairs, extra_tiles):
            """Two attention streams + qk-projection tiles for a later group
            interleaved between super-steps (keeps PE dense while ACT/DVE
            drain)."""
            streams = [attention_closures(hp) for hp in pairs]
            # per extra tile j: [chain qc0] then [chain qc1 + drain]
            extras = []
            qk_state = {}

            def ex_chain0(j):
                ps = psum.tile([P, 2, QW], F32, tag="blk", name="qk_ps")
                qk_state[j] = ps
                qk_chain(j, 0, ps)

            def ex_chain1(j):
                ps = qk_state.pop(j)
                qk_chain(j, 1, ps)
                qk_drain(j, ps)

            for j in extra_tiles:
                extras.append(lambda j=j: ex_chain0(j))
                extras.append(lambda j=j: ex_chain1(j))

            LAG = 3 if extras else 4
            n = len(streams[0][0])
            ei = 0
            for i in range(n + LAG):
                if i < n:
                    for sc_fns, _ in streams:
                        sc_fns[i]()
                if ei < len(extras):
                    extras[ei]()
                    ei += 1
                if i >= LAG:
                    for _, av_fns in streams:
                        av_fns[i - LAG]()
            while ei < len(extras):
                extras[ei]()
                ei += 1

        # ---- schedule ----
        for qc in range(NQC):          # tiles q0..k2 ; all qc=0 chains first
            for j in range(6):
                emit_qk_half(j, qc)
        for tt in range(TT):
            emit_v(tt)
        run_group([0, 1], [6, 7, 8, 9])
        run_group([2, 3], [10, 11])
        run_group([4, 5], [])

    nc.compile()
    return nc


_CACHE = {}


def _get_module():
    if "nc" not in _CACHE:
        _CACHE["nc"] = build_module()
    return _CACHE["nc"]


def _prep_in_maps(x, W_attn, b_attn, T=1024, C=768, n_cores=8):
    bf = ml_dtypes.bfloat16
    CT = C // P
    W = np.asarray(W_attn, dtype=np.float32)
    b = np.asarray(b_attn, dtype=np.float32)

    # q/k tiles interleaved per head pair: j even -> q tile j//2, odd -> k
    wq = W[0:C].reshape(CT, P, CT, P)          # [jq, m, ct, p]
    wk = W[C:2 * C].reshape(CT, P, CT, P)
    wqk_t = np.empty((P, 2 * CT, CT, P), dtype=np.float32)
    for h in range(CT):
        wqk_t[:, 2 * h] = wq[h].transpose(2, 1, 0)       # [p, ct, m]
        wqk_t[:, 2 * h + 1] = wk[h].transpose(2, 1, 0)
    wqk_np = np.ascontiguousarray(
        wqk_t.reshape(P, 2 * C * CT)).astype(bf)

    wv_np = np.ascontiguousarray(
        W[2 * C:3 * C].reshape(C, CT, P).transpose(2, 1, 0)
        .reshape(P, CT * C)).astype(bf)

    bqk_np = np.empty((P, 2 * CT), dtype=np.float32)
    for h in range(CT):
        bqk_np[:, 2 * h] = b[h * P:(h + 1) * P]
        bqk_np[:, 2 * h + 1] = b[C + h * P:C + (h + 1) * P]
    bv_np = np.ascontiguousarray(
        np.tile(b[2 * C:][None, :], (P, 1))).astype(np.float32)

    in_maps = []
    for c in range(n_cores):
        xL_np = np.ascontiguousarray(
            np.asarray(x[c], dtype=np.float32).reshape(T, CT, P)
            .transpose(2, 1, 0).reshape(P, CT * T)).astype(bf)
        in_maps.append({"xL": xL_np, "wqk": wqk_np, "wv": wv_np,
                        "bqk": bqk_np, "bv": bv_np})
    return in_maps


def run(x, W_attn, b_attn, trace=False):
    nc = _get_module()
    in_maps = _prep_in_maps(x, W_attn, b_attn)
    res = bass_utils.run_bass_kernel_spmd(
        nc, in_maps, core_ids=list(range(8)), trace=trace)
    y = np.stack([np.asarray(res.results[c]["yT"]).astype(np.float32).T
                  for c in range(8)])
    return np.ascontiguousarray(y), res


def kernel(x, W_attn, b_attn):
    y, _ = run(x, W_attn, b_attn, trace=False)
    return y


# revision 19
# speedup vs baseline: 1.0217x; 1.0217x over previous
"""Causal ReLU-attention block (qkv proj + per-head attention) on 8 trn2 cores.

Sharding: pure data-parallel over batch (B=8 -> 1 batch element per core).

Schedule (single TileContext, PE-bound):
  1. qk projection tiles for head pairs 0,1  (q0,k0,q1,k1)
  2. v projection (all t-tiles)
  3. attention pairs (0,1)  interleaved with qk tiles for pairs 2,3
  4. attention pairs (2,3)  interleaved with qk tiles for pairs 4,5
  5. attention pairs (4,5)
PSUM->SBUF evictions are load-balanced across ACT and DVE via a cost model;
causal masking of diagonal strips runs on gpsimd. Host side: weights are
pre-laid-out tile-major so every DMA is a large per-partition-contiguous
transfer; y is returned transposed in bf16 and cast on host.
"""

import sys
from contextlib import ExitStack

sys.path.insert(0, "/opt/trn_rl_repo")

import ml_dtypes
import numpy as np

import concourse.bass as bass
import concourse.tile as tile
from concourse import bacc, bass_utils, mybir

P = 128
QW = 512  # t_q chunk width (PSUM bank = 512 fp32)

BF16 = mybir.dt.bfloat16
F32 = mybir.dt.float32
AF = mybir.ActivationFunctionType
ALU = mybir.AluOpType


def build_module(T=1024, C=768, H=12, n_cores=8):
    """Build + compile the per-core Bass module (same program on all cores)."""
    hd = C // H
    assert hd == 64 and H % 2 == 0 and C % P == 0 and T % QW == 0
    CT = C // P            # contraction tiles over C
    TT = T // P            # t tiles
    NQC = T // QW          # q chunks
    VW = C // 2            # v output free-dim chunk width (<=512)
    NHP = H // 2           # head pairs
    scale = 1.0 / float(np.sqrt(hd))

    nc = bacc.Bacc("TRN2", target_bir_lowering=False, debug=False,
                   num_devices=n_cores)

    # q/k weights, tile-major interleaved [q0,k0,q1,k1,...]: [P, 12, CT, P]
    wqk = nc.dram_tensor("wqk", [P, 2 * C * CT], BF16,
                         kind="ExternalInput").ap()
    # v weights [P, CT, C]
    wv = nc.dram_tensor("wv", [P, CT * C], BF16, kind="ExternalInput").ap()
    xL = nc.dram_tensor("xL", [P, CT * T], BF16, kind="ExternalInput").ap()
    bqk = nc.dram_tensor("bqk", [P, 2 * CT], F32, kind="ExternalInput").ap()
    bv = nc.dram_tensor("bv", [P, C], F32, kind="ExternalInput").ap()
    yT = nc.dram_tensor("yT", [C, T], BF16, kind="ExternalOutput").ap()

    wqk3 = wqk.rearrange("p (j ct m) -> p j ct m", j=2 * CT, ct=CT)
    wv3 = wv.rearrange("p (ct o) -> p ct o", ct=CT)
    xL3 = xL.rearrange("p (ct t) -> p ct t", ct=CT)

    with tile.TileContext(nc) as tc, ExitStack() as ctx:
        const = ctx.enter_context(tc.tile_pool(name="const", bufs=1))
        psum = ctx.enter_context(tc.tile_pool(name="psum", bufs=3, space="PSUM"))
        ypsum = ctx.enter_context(tc.tile_pool(name="ypsum", bufs=2, space="PSUM"))
        scb = ctx.enter_context(tc.tile_pool(name="scb", bufs=12))
        ysb = ctx.enter_context(tc.tile_pool(name="ysb", bufs=3))

        wqk_sb = const.tile([P, 2 * CT, CT, P], BF16)
        wv_sb = const.tile([P, CT, C], BF16)
        xt_sb = const.tile([P, CT, T], BF16)
        bqk_sb = const.tile([P, 2 * CT], F32)
        bv_sb = const.tile([P, 2, VW], F32)

        # input staging in need-priority classes, byte-balanced across the
        # two HWDGE queues (the 16 shared DMA engines saturate ~355GB/s
        # aggregate, so order is what matters, not more queues).
        nc.scalar.dma_start(bqk_sb[:], bqk[:])
        nc.sync.dma_start(xt_sb[:, 0:2, 0:QW], xL3[:, 0:2, 0:QW])     # cls1
        nc.scalar.dma_start(wqk_sb[:, 0:1], wqk3[:, 0:1])             # q0
        nc.sync.dma_start(xt_sb[:, 2:4, 0:QW], xL3[:, 2:4, 0:QW])
        nc.scalar.dma_start(xt_sb[:, 4:6, 0:QW], xL3[:, 4:6, 0:QW])
        nc.sync.dma_start(wqk_sb[:, 1:2], wqk3[:, 1:2])               # k0
        nc.scalar.dma_start(wqk_sb[:, 2:3], wqk3[:, 2:3])             # q1
        nc.sync.dma_start(wqk_sb[:, 3:4], wqk3[:, 3:4])               # k1
        nc.scalar.dma_start(xt_sb[:, 0:3, QW:T], xL3[:, 0:3, QW:T])   # cls3
        nc.sync.dma_start(xt_sb[:, 3:6, QW:T], xL3[:, 3:6, QW:T])
        nc.scalar.dma_start(wv_sb[:, :, 0:VW], wv3[:, :, 0:VW])       # cls4
        nc.sync.dma_start(wv_sb[:, :, VW:C], wv3[:, :, VW:C])
        nc.scalar.dma_start(wqk_sb[:, 4:8], wqk3[:, 4:8])             # cls5
        nc.sync.dma_start(wqk_sb[:, 8:12], wqk3[:, 8:12])
        nc.sync.dma_start(bv_sb[:], bv.rearrange("p (oc v) -> p oc v", oc=2))

        # PE warm-up: HAM starts at K=4/8 (1.2 GHz) and needs ~3.4us of
        # sustained activity to unthrottle; burn the initial DMA wait on
        # dummy matmuls so real chains run at 2.4 GHz from the start.
        wu_src = const.tile([P, 256], BF16)
        nc.gpsimd.memset(wu_src[:], 0.0)
        wu_ps = ypsum.tile([P, QW], F32, tag="y", name="wu_ps")
        for _ in range(16):
            nc.tensor.matmul(wu_ps[0:1, 0:256], wu_src[:, 0:1],
                             wu_src[:], start=True, stop=True)

        qkT = const.tile([P, 2 * CT, T], BF16)   # tile j: q_h=j//2 (even) / k (odd)
        vsb = const.tile([P, TT, C], BF16)       # v in natural [t, o] layout

        # ---- ACT/DVE eviction load balancer (ns cost model, trn2 errata) ----
        est = {"a": 0.0, "d": 0.0}

        def _pick(fd):
            a_cost = (172.0 + fd) / 1.2
            d_cost = (120.0 + fd) / 0.96
            if est["a"] + a_cost <= est["d"] + d_cost:
                est["a"] += a_cost
                return "a"
            est["d"] += d_cost
            return "d"

        def relu_evict(dst, src, fd):
            # relu(scale * s): PSUM -> SBUF bf16
            if _pick(fd) == "a":
                nc.scalar.activation(dst, src, AF.Relu, scale=scale)
            else:
                nc.vector.tensor_scalar(dst, src, scale, 0.0, ALU.mult, ALU.max)

        def y_evict(dst, src):
            if _pick(QW) == "a":
                nc.scalar.activation(dst, src, AF.Copy)
            else:
                nc.vector.tensor_copy(dst, src)

        def qk_drain(j, ps):
            flat = ps[:, :NQC].rearrange("p a b -> p (a b)")
            if _pick(NQC * QW) == "a":
                nc.scalar.activation(qkT[:, j], flat, AF.Identity,
                                     bias=bqk_sb[:, j:j + 1])
            else:
                nc.vector.tensor_tensor(
                    qkT[:, j], flat,
                    bqk_sb[:, j:j + 1].to_broadcast((P, NQC * QW)), ALU.add)

        # ---- projections ----
        def qk_chain(j, qc, ps, slot=None):
            slot = qc if slot is None else slot
            for ct in range(CT):
                nc.tensor.matmul(
                    ps[:, slot],
                    wqk_sb[:, j, ct],
                    xt_sb[:, ct, qc * QW:(qc + 1) * QW],
                    start=(ct == 0), stop=(ct == CT - 1),
                )

        def qk_drain_half(j, qc, ps):
            dst = qkT[:, j, qc * QW:(qc + 1) * QW]
            if _pick(QW) == "a":
                nc.scalar.activation(dst, ps[:, 0], AF.Identity,
                                     bias=bqk_sb[:, j:j + 1])
            else:
                nc.vector.tensor_tensor(
                    dst, ps[:, 0],
                    bqk_sb[:, j:j + 1].to_broadcast((P, QW)), ALU.add)

        def emit_qk_half(j, qc):
            # one qc chunk per PSUM tile so the qc=0 chains of all four
            # phase-1 tiles can run before x's second half has landed
            ps = psum.tile([P, 2, QW], F32, tag="blk", name="qk_ps")
            qk_chain(j, qc, ps, slot=0)
            qk_drain_half(j, qc, ps)

        def emit_v(tt, ocs=(0, 1)):
            ps = psum.tile([P, 2, QW], F32, tag="blk", name="v_ps")
            for ct in range(CT):
                for oc in ocs:
                    nc.tensor.matmul(
                        ps[:, oc, :VW],
                        xt_sb[:, ct, tt * P:(tt + 1) * P],
                        wv_sb[:, ct, oc * VW:(oc + 1) * VW],
                        start=(ct == 0), stop=(ct == CT - 1),
                    )
            nc.vector.tensor_tensor(
                vsb[:, tt].rearrange("p (oc v) -> p oc v", oc=2),
                ps[:, :, :VW], bv_sb[:], ALU.add)
            est["d"] += (120.0 + C) / 0.96

        # ---- attention ----
        def attention_closures(hp):
            """Score / att@v closures per block step for one head pair; the
            interleaver runs att@v LAG super-steps behind its scores."""
            items = []
            for qc in range(NQC):
                kb_hi = min((qc * QW + QW - 1) // P, TT - 1)
                for kb in range(kb_hi + 1):
                    items.append((qc, kb, kb_hi))
            state = {"s": {}, "y": {}}
            sc_fns, av_fns = [], []

            def sc(i, qc, kb, kb_hi):
                delta = max(kb * P - qc * QW, 0)   # first valid t_q col
                sp = psum.tile([P, 2, QW], F32, tag="blk", name="s_ps")
                for h, ppos in ((0, (0, 0)), (1, (64, 0))):
                    nc.tensor.matmul(
                        sp[:, h, delta:QW],
                        qkT[h * 64:(h + 1) * 64, 2 * hp + 1,
                            kb * P:(kb + 1) * P],
                        qkT[h * 64:(h + 1) * 64, 2 * hp,
                            qc * QW + delta:(qc + 1) * QW],
                        start=True, stop=True, tile_position=ppos,
                    )
                s = scb.tile([P, 2, QW], BF16, tag="s")
                relu_evict(s[:, :, delta:QW], sp[:, :, delta:QW],
                           2 * (QW - delta))
                if kb * P >= qc * QW:   # diagonal strip: causal mask on the
                    # first P cols only (row p can only mask j' < p < P)
                    nc.gpsimd.affine_select(
                        s[:, :, delta:delta + P],
                        s[:, :, delta:delta + P],
                        pattern=[[0, 2], [1, P]],
                        compare_op=ALU.is_ge, fill=0.0,
                        base=0, channel_multiplier=-1,
                    )
                state["s"][i] = s

            def av(i, qc, kb, kb_hi):
                if kb == 0:
                    state["y"][qc] = ypsum.tile([P, QW], F32, tag="y",
                                                name="yp")
                yp = state["y"][qc]
                delta = max(kb * P - qc * QW, 0)
                s = state["s"].pop(i)
                # the two heads accumulate into disjoint partition ranges of
                # one bank; each runs its own start/stop group
                nc.tensor.matmul(
                    yp[0:64, delta:QW], vsb[:, kb, hp * P:hp * P + 64],
                    s[:, 0, delta:QW],
                    start=(kb == 0), stop=(kb == kb_hi),
                    tile_position=(0, 0), skip_group_check=True,
                )
                nc.tensor.matmul(
                    yp[64:128, delta:QW],
                    vsb[:, kb, hp * P + 64:hp * P + 128],
                    s[:, 1, delta:QW],
                    start=(kb == 0), stop=(kb == kb_hi),
                    tile_position=(0, 64), skip_group_check=True,
                )
                if kb == kb_hi:
                    yp = state["y"].pop(qc)
                    yt = ysb.tile([P, QW], BF16, tag="yt")
                    y_evict(yt[:], yp[:])
                    nc.sync.dma_start(
                        yT[hp * P:(hp + 1) * P, qc * QW:(qc + 1) * QW],
                        yt[:])

            for i, (qc, kb, kb_hi) in enumerate(items):
                sc_fns.append(
                    lambda i=i, qc=qc, kb=kb, kb_hi=kb_hi: sc(i, qc, kb, kb_hi))
                av_fns.append(
                    lambda i=i, qc=qc, kb=kb, kb_hi=kb_hi: av(i, qc, kb, kb_hi))
            return sc_fns, av_fns

        def run_group(pairs, extra_tiles):
            """Two attention streams + qk-projection tiles for a later group
            interleaved between super-steps (keeps PE dense while ACT/DVE
            drain)."""
            streams = [attention_closures(hp) for hp in pairs]
            # per extra tile j: [chain qc0] then [chain qc1 + drain]
            extras = []
            qk_state = {}

            def ex_chain0(j):
                ps = psum.tile([P, 2, QW], F32, tag="blk", name="qk_ps")
                qk_state[j] = ps
                qk_chain(j, 0, ps)

            def ex_chain1(j):
                ps = qk_state.pop(j)
                qk_chain(j, 1, ps)
                qk_drain(j, ps)

            for j in extra_tiles:
                extras.append(lambda j=j: ex_chain0(j))
                extras.append(lambda j=j: ex_chain1(j))

            LAG = 3 if extras else 4
            n = len(streams[0][0])
            ei = 0
            for i in range(n + LAG):
                if i < n:
                    for sc_fns, _ in streams:
                        sc_fns[i]()
                if ei < len(extras):
                    extras[ei]()
                    ei += 1
                if i >= LAG:
                    for _, av_fns in streams:
                        av_fns[i - LAG]()
            while ei < len(extras):
                extras[ei]()
                ei += 1

        # ---- schedule ----
        for qc in range(NQC):          # q0,k0,q1,k1 ; qc=0 chains first
            for j in range(4):
                emit_qk_half(j, qc)
        for tt in range(TT):
            emit_v(tt)
        run_group([0, 1], [4, 5, 6, 7])
        run_group([2, 3], [8, 9, 10, 11])
        run_group([4, 5], [])

    nc.compile()
    return nc


_CACHE = {}


def _get_module():
    if "nc" not in _CACHE:
        _CACHE["nc"] = build_module()
    return _CACHE["nc"]


def _prep_in_maps(x, W_attn, b_attn, T=1024, C=768, n_cores=8):
    bf = ml_dtypes.bfloat16
    CT = C // P
    W = np.asarray(W_attn, dtype=np.float32)
    b = np.asarray(b_attn, dtype=np.float32)

    # q/k tiles interleaved per head pair: j even -> q tile j//2, odd -> k
    wq = W[0:C].reshape(CT, P, CT, P)          # [jq, m, ct, p]
    wk = W[C:2 * C].reshape(CT, P, CT, P)
    wqk_t = np.empty((P, 2 * CT, CT, P), dtype=np.float32)
    for h in range(CT):
        wqk_t[:, 2 * h] = wq[h].transpose(2, 1, 0)       # [p, ct, m]
        wqk_t[:, 2 * h + 1] = wk[h].transpose(2, 1, 0)
    wqk_np = np.ascontiguousarray(
        wqk_t.reshape(P, 2 * C * CT)).astype(bf)

    wv_np = np.ascontiguousarray(
        W[2 * C:3 * C].reshape(C, CT, P).transpose(2, 1, 0)
        .reshape(P, CT * C)).astype(bf)

    bqk_np = np.empty((P, 2 * CT), dtype=np.float32)
    for h in range(CT):
        bqk_np[:, 2 * h] = b[h * P:(h + 1) * P]
        bqk_np[:, 2 * h + 1] = b[C + h * P:C + (h + 1) * P]
    bv_np = np.ascontiguousarray(
        np.tile(b[2 * C:][None, :], (P, 1))).astype(np.float32)

    in_maps = []
    for c in range(n_cores):
        xL_np = np.ascontiguousarray(
            np.asarray(x[c], dtype=np.float32).reshape(T, CT, P)
            .transpose(2, 1, 0).reshape(P, CT * T)).astype(bf)
        in_maps.append({"xL": xL_np, "wqk": wqk_np, "wv": wv_np,
                        "bqk": bqk_np, "bv": bv_np})
    return in_maps


def run(x, W_attn, b_attn, trace=False):
    nc = _get_module()
    in_maps = _prep_in_maps(x, W_attn, b_attn)
    res = bass_utils.run_bass_kernel_spmd(
        nc, in_maps, core_ids=list(range(8)), trace=trace)
    y = np.stack([np.asarray(res.results[c]["yT"]).astype(np.float32).T
                  for c in range(8)])
    return np.ascontiguousarray(y), res


def kernel(x, W_attn, b_attn):
    y, _ = run(x, W_attn, b_attn, trace=False)
    return y
